# revision 16
# baseline (speedup 1.0000x reference)
"""Trainium2 Bass kernel for an attention block with a non-standard
(query-axis) softmax and causal mask.  fp8 DoubleRow version.

Math per batch element b (T=2048 tokens, C=K=V=512):
    q = x @ Wq.T + bq ; k = x @ Wk.T + bk ; v = x @ Wv.T + bv
    logits[j, i] = q[j] . k[i]                     (j=query, i=key)
    masked = -inf where i > j
    probs = softmax(masked / sqrt(512), axis=j)    <-- softmax over QUERY axis
    read[j] = sum_i probs[j, i] * v[i]
    out = concat(x, read)                          [T, 1024]

Distribution: pure data-parallel, batch b -> core b (8 batches, 8 cores),
weights replicated, no collectives.  The passthrough half of the output is
assembled on the host (np.concatenate); the device computes only read.

All matmuls run in fp8(e4m3) with perf_mode=DoubleRow: one instruction
contracts TWO 128-row slabs (lhsT [128,2,M], rhs [128,2,N]) at the same
rate a bf16 matmul contracts one -- 2x effective PE throughput (measured
222ns per [128out x 512free x 256contract] on HW).

Scale management (e4m3: max 240, min normal 2^-6):
  - X cast to fp8 directly (rms 1).  Weights scaled by 32 (rms 0.64); the
    1/32 is folded into the PSUM-evacuation affines.
  - Q,K stored as fp8 2q, 2k (rms 0.9); the extra 2*2 and the softmax
    1/sqrt(512) fold into the ACT exp scale.
  - The softmax normalizer 1/sum_j e spans [1/2048, 1] across key rows i.
    That 11-octave range is split evenly between the two read-matmul
    operands: E~[i,j] = e[i,j]*2^-t_i (via a static per-partition bias
    -t_i*ln2 added inside the exp) and V~[i,v] = v[i,v] / sum_j E~[i,j].
    Then E~ . V~ == probs . v exactly, and both operands sit near the
    middle of the fp8 range (t_i = round(log2(2.39*n_i)/2), n_i = 2048-i).

Engine budget (PSUM can only be read by ACT/DVE; GPSIMD is SBUF-only):
  PE ~56us (248 DR matmuls + warmups), ACT ~35us (Q/K affines via
  Identity+bias, 24 exps with accum), DVE ~39us (K affines, V bias-stt,
  triangular mask adds on the 128-wide diagonal strips, reciprocals,
  PSUM->SBUF output copies), GPSIMD ~11us (V~ scaling into fp8, part sums,
  zero strips for the even-jt diagonal pairs, nothing touching PSUM).

Phase 2 (logits+exp) and phase 3 (read) are emission-interleaved so the
in-order PE queue never waits long on the ACT exp pipeline; the last two
read rows' accumulation chains are split so only one pair of matmuls
remains after the final exp.
"""

import math

import numpy as np
import ml_dtypes

P = 128
B, T, C = 8, 2048, 512
NT = T // P     # 16 row tiles
NCORES = 8
NEG = -1e30
SEXP = 1.0 / (4.0 * math.sqrt(512.0))

_BUILT = None


def _build_nc():
    import concourse.mybir as mybir
    import concourse.tile as tile
    from concourse import bacc

    f32 = mybir.dt.float32
    bf16 = mybir.dt.bfloat16
    fp8 = mybir.dt.float8e4
    AF = mybir.ActivationFunctionType
    ALU = mybir.AluOpType
    DR = mybir.MatmulPerfMode.DoubleRow

    nc = bacc.Bacc("TRN2", target_bir_lowering=False, debug=False,
                   num_devices=NCORES)

    xt_d = nc.dram_tensor("xt", [C, T], fp8, kind="ExternalInput")
    wq_d = nc.dram_tensor("wq", [P, 2048], fp8, kind="ExternalInput")
    wk_d = nc.dram_tensor("wk", [P, 2048], fp8, kind="ExternalInput")
    wv_d = nc.dram_tensor("wv", [P, 2048], fp8, kind="ExternalInput")
    bqk_d = nc.dram_tensor("bqk", [P, 8], f32, kind="ExternalInput")
    bexp_d = nc.dram_tensor("bexp", [P, NT], f32, kind="ExternalInput")
    mask_d = nc.dram_tensor("mask", [P, P], bf16, kind="ExternalInput")
    bvf_d = nc.dram_tensor("bvf", [P, 2 * C], bf16, kind="ExternalInput")
    out_d = nc.dram_tensor("out", [T, C], f32, kind="ExternalOutput")

    with tile.TileContext(nc) as tc:
        with (
            tc.tile_pool(name="const", bufs=1) as cpool,
            tc.tile_pool(name="w", bufs=1) as wpool,
            tc.tile_pool(name="xt", bufs=1) as xtpool,
            tc.tile_pool(name="qt", bufs=1) as qtpool,
            tc.tile_pool(name="kt", bufs=1) as ktpool,
            tc.tile_pool(name="v", bufs=1) as vpool,
            tc.tile_pool(name="vp", bufs=1) as vppool,
            tc.tile_pool(name="et", bufs=1) as etpool,
            tc.tile_pool(name="small", bufs=16) as spool,
            tc.tile_pool(name="ostage", bufs=3) as ospool,
            tc.tile_pool(name="psq", bufs=2, space="PSUM") as psq,   # 2x1024
            tc.tile_pool(name="psl", bufs=2, space="PSUM") as psl,   # 2x512
            tc.tile_pool(name="pso", bufs=2, space="PSUM") as pso,   # 2x512
        ):
            # --- loads, in first-use order (single HWDGE queue is FIFO) ---
            bqk_t = cpool.tile([P, 8], f32, name="bqk_t")
            nc.sync.dma_start(bqk_t[:], bqk_d[:])
            bexp_t = cpool.tile([P, NT], f32, name="bexp_t")
            nc.sync.dma_start(bexp_t[:], bexp_d[:])
            mask_t = cpool.tile([P, P], bf16, name="mask_t")
            nc.sync.dma_start(mask_t[:], mask_d[:])
            bvf_t = cpool.tile([P, 2 * C], bf16, name="bvf_t")
            nc.sync.dma_start(bvf_t[:], bvf_d[:])

            # PE warm-up: junk bf16 matmuls with no DMA dependency so the
            # HAM clock gate is at full rate when real work arrives.
            warm_src = cpool.tile([P, C + P], bf16, name="warm_src")
            nc.vector.memset(warm_src[:], 0.0)
            ps_warm = pso.tile([P, 512], f32, name="ps_warm", tag="pso")
            for _ in range(14):
                nc.tensor.matmul(ps_warm[:], warm_src[:, C:C + P],
                                 warm_src[:, 0:C], start=True, stop=True)

            # weights / X^T.  Pair layout: index [c_lo, s, k] with the
            # contraction row c = cp*256 + s*128 + c_lo.  One DMA per SBUF
            # tile / row-block: dma_start issue on the sync engine is slow
            # (~1us each), so few big transfers beat many small ones.
            wq2 = []
            for cp in range(2):
                t_ = wpool.tile([P, 2, C], fp8, name=f"wq{cp}", tag=f"wq{cp}")
                nc.sync.dma_start(t_[:, :, :],
                                  wq_d[:, cp * 1024:(cp + 1) * 1024])
                wq2.append(t_)
            wk2 = []
            for cp in range(2):
                t_ = wpool.tile([P, 2, C], fp8, name=f"wk{cp}", tag=f"wk{cp}")
                nc.sync.dma_start(t_[:, :, :],
                                  wk_d[:, cp * 1024:(cp + 1) * 1024])
                wk2.append(t_)
            xt2 = [xtpool.tile([P, 2, T], fp8, name=f"xt{cp}", tag=f"xt{cp}")
                   for cp in range(2)]
            for cp in range(2):
                for s in range(2):
                    r0 = (2 * cp + s) * P
                    nc.sync.dma_start(xt2[cp][:, s, :], xt_d[r0:r0 + P, :])
            wv2 = []
            for cp in range(2):
                t_ = wpool.tile([P, 2, C], fp8, name=f"wv{cp}", tag=f"wv{cp}")
                nc.sync.dma_start(t_[:, :, :],
                                  wv_d[:, cp * 1024:(cp + 1) * 1024])
                wv2.append(t_)

            # --- Phase 1a: Q^T, K^T in [k, 2, t] fp8 pair layout ---
            # Q^T psum = 32*q_raw; evac: *1/16 + 2*b -> fp8 2q (rms ~0.9).
            # Two 512-col chunks share one [128,1024] psum tile so a single
            # 1024-wide affine evacuates both (half the evac ops, and the
            # PE stays dense enough to hold the HAM clock at full rate).
            # jcp-outer order: the first half of phase 1a touches only
            # X^T[:, 0:1024], hiding the rest of the X^T load.
            qt2 = [qtpool.tile([P, 2, T], fp8, name=f"qt{kp}", tag=f"qt{kp}")
                   for kp in range(2)]
            kt2 = [ktpool.tile([P, 2, T], fp8, name=f"kt{kp}", tag=f"kt{kp}")
                   for kp in range(2)]
            for jcp in range(2):
                jp = slice(jcp * 1024, (jcp + 1) * 1024)
                for kt in range(4):
                    ksl = slice(kt * P, (kt + 1) * P)
                    ps = psq.tile([P, 1024], f32, name="psq1", tag="psq")
                    for h in range(2):
                        js = slice((2 * jcp + h) * 512,
                                   (2 * jcp + h + 1) * 512)
                        for cp in range(2):
                            nc.tensor.matmul(ps[:, h * 512:(h + 1) * 512],
                                             wq2[cp][:, :, ksl],
                                             xt2[cp][:, :, js],
                                             start=(cp == 0), stop=(cp == 1),
                                             perf_mode=DR)
                    nc.scalar.activation(qt2[kt // 2][:, kt % 2, jp], ps[:],
                                         AF.Identity,
                                         bias=bqk_t[:, kt:kt + 1],
                                         scale=1.0 / 16.0)
                    ps2 = psq.tile([P, 1024], f32, name="psq2", tag="psq")
                    for h in range(2):
                        js = slice((2 * jcp + h) * 512,
                                   (2 * jcp + h + 1) * 512)
                        for cp in range(2):
                            nc.tensor.matmul(ps2[:, h * 512:(h + 1) * 512],
                                             wk2[cp][:, :, ksl],
                                             xt2[cp][:, :, js],
                                             start=(cp == 0), stop=(cp == 1),
                                             perf_mode=DR)
                    nc.vector.tensor_scalar(kt2[kt // 2][:, kt % 2, jp],
                                            ps2[:], 1.0 / 16.0,
                                            bqk_t[:, 4 + kt:5 + kt],
                                            ALU.mult, ALU.add)

            # --- Phase 1b: V in natural [t, v] layout, bf16 (true scale) ---
            # Paired the same way: two token tiles per psum/evac.
            v2_sb = []
            for tp in range(8):
                psV = psq.tile([P, 1024], f32, name="psv", tag="psq")
                for h in range(2):
                    tsl = slice((2 * tp + h) * P, (2 * tp + h + 1) * P)
                    for cp in range(2):
                        nc.tensor.matmul(psV[:, h * 512:(h + 1) * 512],
                                         xt2[cp][:, :, tsl],
                                         wv2[cp][:, :, :],
                                         start=(cp == 0), stop=(cp == 1),
                                         perf_mode=DR)
                vt = vpool.tile([P, 2 * C], bf16, name=f"v{tp}", tag=f"v{tp}")
                nc.vector.scalar_tensor_tensor(vt[:], psV[:], 1.0 / 32.0,
                                               bvf_t[:], ALU.mult, ALU.add)
                v2_sb.append(vt)

            # E~ pair tiles [i_lo, s, j]; key row i = 256r + 128s + i_lo.
            et2 = [etpool.tile([P, 2, T], fp8, name=f"et{r}", tag=f"et{r}")
                   for r in range(8)]
            # Zero the garbage diagonal strips read by even-jt read chains:
            # pair r = jt/2 slab 1 is key tile it = jt+1, whose columns
            # [128jt, 128(jt+1)) are fully masked and never written by exp.
            for jt in range(0, NT, 2):
                nc.vector.memset(et2[jt // 2][:, 1, jt * P:(jt + 1) * P], 0.0)
            vp2 = [vppool.tile([P, 2, C], fp8, name=f"vp{r}", tag=f"vp{r}")
                   for r in range(8)]

            # --- Phase 2 per key tile: masked logits + exp + row sums ---
            def p2_it(it):
                isl = slice(it * P, (it + 1) * P)
                parts = []
                w0 = it * P
                # 512-wide psum chunks, 512-aligned in absolute column space
                # (matmul writes must stay within one psum bank).
                for base in range(w0 // 512 * 512, T, 512):
                    lo = max(w0, base)
                    hi = base + 512
                    pslt = psl.tile([P, 512], f32, name="psl", tag="psl")
                    for cp in range(2):
                        nc.tensor.matmul(pslt[:, lo - base:512],
                                         kt2[cp][:, :, isl],
                                         qt2[cp][:, :, lo:hi],
                                         start=(cp == 0), stop=(cp == 1),
                                         perf_mode=DR)
                    if lo == w0:  # triangular mask on the diagonal strip
                        nc.vector.tensor_add(pslt[:, lo - base:lo - base + P],
                                             pslt[:, lo - base:lo - base + P],
                                             mask_t[:])
                    part = spool.tile([P, 1], f32, name="part", tag="part")
                    nc.scalar.activation(et2[it // 2][:, it % 2, lo:hi],
                                         pslt[:, lo - base:512], AF.Exp,
                                         bias=bexp_t[:, it:it + 1],
                                         scale=SEXP, accum_out=part[:])
                    parts.append(part)
                s_t = parts[0]
                for p_ in parts[1:]:
                    ns = spool.tile([P, 1], f32, name="s_t", tag="s_t")
                    nc.vector.tensor_add(ns[:], s_t[:], p_[:])
                    s_t = ns
                r_t = spool.tile([P, 1], f32, name="r_t", tag="r_t")
                nc.vector.reciprocal(r_t[:], s_t[:])
                nc.scalar.activation(
                    vp2[it // 2][:, it % 2, :],
                    v2_sb[it // 2][:, (it % 2) * 512:(it % 2 + 1) * 512],
                    AF.Copy, scale=r_t[:, 0:1])

            # --- Phase 3 per query tile: read[jt] = E~^T pairs @ V~ ---
            ps_o = {}

            def p3_mm(jt, rlo, rhi):
                jsl = slice(jt * P, (jt + 1) * P)
                R = jt // 2 + 1
                if jt not in ps_o:
                    ps_o[jt] = pso.tile([P, 512], f32, name="pso", tag="pso")
                for r in range(rlo, rhi):
                    nc.tensor.matmul(ps_o[jt][:], et2[r][:, :, jsl],
                                     vp2[r][:, :, :],
                                     start=(r == 0), stop=(r == R - 1),
                                     perf_mode=DR)

            def p3_out(jt):
                jsl = slice(jt * P, (jt + 1) * P)
                ost = ospool.tile([P, 512], f32, name="ost", tag="ost")
                nc.vector.tensor_copy(ost[:], ps_o[jt][:])
                # software-DGE queues: keeps the HWDGE path + sync engine
                # free for input loads
                nc.gpsimd.dma_start(out_d[jsl, :], ost[:])

            # Two-iteration lag between p2 producers and p3 consumers so the
            # exp -> reciprocal -> V~-scale chain latency stays hidden
            # behind queued PE work.
            p2_it(0)
            p2_it(1)
            for k in range(2, 14):
                p2_it(k)
                p3_mm(k - 2, 0, (k - 2) // 2 + 1)
                p3_out(k - 2)
            # k=13 done: jt 12,13 fully ready; jt 14/15 all but last pair
            p3_mm(12, 0, 7)
            p3_out(12)
            p3_mm(13, 0, 7)
            p3_out(13)
            p3_mm(14, 0, 7)
            p2_it(14)
            p3_mm(15, 0, 7)
            p2_it(15)
            p3_mm(14, 7, 8)
            p3_out(14)
            p3_mm(15, 7, 8)
            p3_out(15)

    nc.compile()
    return nc


def _get_built():
    global _BUILT
    if _BUILT is None:
        _BUILT = _build_nc()
    return _BUILT


def _make_in_maps(input, Wq, bq, Wk, bk, Wv, bv):
    fp8 = ml_dtypes.float8_e4m3
    bf = ml_dtypes.bfloat16

    input = np.asarray(input, np.float32)
    Wq = np.asarray(Wq, np.float32)
    bq = np.asarray(bq, np.float32)
    Wk = np.asarray(Wk, np.float32)
    bk = np.asarray(bk, np.float32)
    Wv = np.asarray(Wv, np.float32)
    bv = np.asarray(bv, np.float32)

    def pack_w(W):
        # [128, 2048] fp8: [c_lo, cp*1024 + s*512 + k], c = cp*256+s*128+c_lo
        Wt = (W * 32.0).T.reshape(2, 2, P, C)          # [cp, s, c_lo, k]
        return np.ascontiguousarray(
            Wt.transpose(2, 0, 1, 3).reshape(P, 2048)).astype(fp8)

    wq = pack_w(Wq)
    wk = pack_w(Wk)
    wv = pack_w(Wv)

    bqk = np.empty((P, 8), np.float32)
    for kt in range(4):
        bqk[:, kt] = 2.0 * bq[kt * P:(kt + 1) * P]
        bqk[:, 4 + kt] = 2.0 * bk[kt * P:(kt + 1) * P]

    # per-key-row power-of-2 softmax split: t_i = round(log2(2.39 n_i)/2)
    i_idx = np.arange(T)
    n_i = (T - i_idx).astype(np.float64)
    t_i = np.round(0.5 * np.log2(2.39 * n_i)).astype(np.float32)
    bexp = np.ascontiguousarray(
        (-t_i * math.log(2.0)).reshape(NT, P).T).astype(np.float32)

    pp = np.arange(P)[:, None]
    xx = np.arange(P)[None, :]
    mask = np.where(xx < pp, NEG, 0.0).astype(bf)

    bvf = np.ascontiguousarray(
        np.broadcast_to(np.tile(bv, 2).astype(bf), (P, 2 * C)))

    in_maps = []
    for b in range(B):
        in_maps.append({
            "xt": np.ascontiguousarray(input[b].T).astype(fp8),
            "wq": wq, "wk": wk, "wv": wv,
            "bqk": bqk, "bexp": bexp, "mask": mask, "bvf": bvf,
        })
    return in_maps


def kernel(input, Wq, bq, Wk, bk, Wv, bv, _trace=False):
    from concourse.bass_utils import run_bass_kernel_spmd

    nc = _get_built()
    in_maps = _make_in_maps(input, Wq, bq, Wk, bk, Wv, bv)
    res = run_bass_kernel_spmd(nc, in_maps, core_ids=list(range(NCORES)),
                               trace=_trace)
    input = np.asarray(input, np.float32)
    out = np.empty((B, T, 2 * C), np.float32)
    for b in range(B):
        out[b, :, 0:C] = input[b]
        out[b, :, C:] = res.results[b]["out"]
    if _trace:
        kernel.last_result = res
    return out


# revision 17
# speedup vs baseline: 1.0790x; 1.0790x over previous
"""Trainium2 Bass kernel for an attention block with a non-standard
(query-axis) softmax and causal mask.  fp8 DoubleRow version.

Math per batch element b (T=2048 tokens, C=K=V=512):
    q = x @ Wq.T + bq ; k = x @ Wk.T + bk ; v = x @ Wv.T + bv
    logits[j, i] = q[j] . k[i]                     (j=query, i=key)
    masked = -inf where i > j
    probs = softmax(masked / sqrt(512), axis=j)    <-- softmax over QUERY axis
    read[j] = sum_i probs[j, i] * v[i]
    out = concat(x, read)                          [T, 1024]

Distribution: pure data-parallel, batch b -> core b (8 batches, 8 cores),
weights replicated, no collectives.  The passthrough half of the output is
assembled on the host (np.concatenate); the device computes only read.

All matmuls run in fp8(e4m3) with perf_mode=DoubleRow: one instruction
contracts TWO 128-row slabs (lhsT [128,2,M], rhs [128,2,N]) at the same
rate a bf16 matmul contracts one -- 2x effective PE throughput (measured
222ns per [128out x 512free x 256contract] on HW).

Scale management (e4m3: max 240, min normal 2^-6):
  - X cast to fp8 directly (rms 1).  Weights scaled by 32 (rms 0.64); the
    1/32 is folded into the PSUM-evacuation affines.
  - Q,K stored as fp8 2q, 2k (rms 0.9); the extra 2*2 and the softmax
    1/sqrt(512) fold into the ACT exp scale.
  - The softmax normalizer 1/sum_j e spans [1/2048, 1] across key rows i.
    That 11-octave range is split evenly between the two read-matmul
    operands: E~[i,j] = e[i,j]*2^-t_i (via a static per-partition bias
    -t_i*ln2 added inside the exp) and V~[i,v] = v[i,v] / sum_j E~[i,j].
    Then E~ . V~ == probs . v exactly, and both operands sit near the
    middle of the fp8 range (t_i = round(log2(2.39*n_i)/2), n_i = 2048-i).

Engine budget (PSUM can only be read by ACT/DVE; GPSIMD is SBUF-only):
  PE ~56us (248 DR matmuls + warmups), ACT ~35us (Q/K affines via
  Identity+bias, 24 exps with accum), DVE ~39us (K affines, V bias-stt,
  triangular mask adds on the 128-wide diagonal strips, reciprocals,
  PSUM->SBUF output copies), GPSIMD ~11us (V~ scaling into fp8, part sums,
  zero strips for the even-jt diagonal pairs, nothing touching PSUM).

Phase 2 (logits+exp) and phase 3 (read) are emission-interleaved so the
in-order PE queue never waits long on the ACT exp pipeline; the last two
read rows' accumulation chains are split so only one pair of matmuls
remains after the final exp.
"""

import math

import numpy as np
import ml_dtypes

P = 128
B, T, C = 8, 2048, 512
NT = T // P     # 16 row tiles
NCORES = 8
NEG = -1e30
SEXP = 1.0 / (4.0 * math.sqrt(512.0))

_BUILT = None


def _build_nc():
    import concourse.mybir as mybir
    import concourse.tile as tile
    from concourse import bacc

    f32 = mybir.dt.float32
    bf16 = mybir.dt.bfloat16
    fp8 = mybir.dt.float8e4
    AF = mybir.ActivationFunctionType
    ALU = mybir.AluOpType
    DR = mybir.MatmulPerfMode.DoubleRow

    nc = bacc.Bacc("TRN2", target_bir_lowering=False, debug=False,
                   num_devices=NCORES)

    xt_d = nc.dram_tensor("xt", [C, T], fp8, kind="ExternalInput")
    wq_d = nc.dram_tensor("wq", [P, 2048], fp8, kind="ExternalInput")
    wk_d = nc.dram_tensor("wk", [P, 2048], fp8, kind="ExternalInput")
    wv_d = nc.dram_tensor("wv", [P, 2048], fp8, kind="ExternalInput")
    bqk_d = nc.dram_tensor("bqk", [P, 8], f32, kind="ExternalInput")
    bexp_d = nc.dram_tensor("bexp", [P, NT], f32, kind="ExternalInput")
    mask_d = nc.dram_tensor("mask", [P, P], bf16, kind="ExternalInput")
    bvf_d = nc.dram_tensor("bvf", [P, 2 * C], bf16, kind="ExternalInput")
    out_d = nc.dram_tensor("out", [T, C], f32, kind="ExternalOutput")

    with tile.TileContext(nc) as tc:
        with (
            tc.tile_pool(name="const", bufs=1) as cpool,
            tc.tile_pool(name="w", bufs=1) as wpool,
            tc.tile_pool(name="xt", bufs=1) as xtpool,
            tc.tile_pool(name="qt", bufs=1) as qtpool,
            tc.tile_pool(name="kt", bufs=1) as ktpool,
            tc.tile_pool(name="v", bufs=1) as vpool,
            tc.tile_pool(name="vp", bufs=1) as vppool,
            tc.tile_pool(name="et", bufs=1) as etpool,
            tc.tile_pool(name="small", bufs=16) as spool,
            tc.tile_pool(name="ostage", bufs=3) as ospool,
            tc.tile_pool(name="psq", bufs=2, space="PSUM") as psq,   # 2x1024
            tc.tile_pool(name="psl", bufs=2, space="PSUM") as psl,   # 2x512
            tc.tile_pool(name="pso", bufs=2, space="PSUM") as pso,   # 2x512
        ):
            # --- loads ---
            # dma_start issue costs ~0.6us of engine time each, so the
            # issues are split across both HWDGE engines (sync + scalar)
            # and ordered most-urgent-first on each.
            xt2 = [xtpool.tile([P, 2, T], fp8, name=f"xt{cp}", tag=f"xt{cp}")
                   for cp in range(2)]
            for cp in range(2):
                for s in range(2):
                    r0 = (2 * cp + s) * P
                    nc.sync.dma_start(xt2[cp][:, s, :], xt_d[r0:r0 + P, :])
            wq2 = []
            for cp in range(2):
                t_ = wpool.tile([P, 2, C], fp8, name=f"wq{cp}", tag=f"wq{cp}")
                nc.scalar.dma_start(t_[:, :, :],
                                    wq_d[:, cp * 1024:(cp + 1) * 1024])
                wq2.append(t_)
            wk2 = []
            for cp in range(2):
                t_ = wpool.tile([P, 2, C], fp8, name=f"wk{cp}", tag=f"wk{cp}")
                nc.scalar.dma_start(t_[:, :, :],
                                    wk_d[:, cp * 1024:(cp + 1) * 1024])
                wk2.append(t_)
            bqk_t = cpool.tile([P, 8], f32, name="bqk_t")
            nc.scalar.dma_start(bqk_t[:], bqk_d[:])
            wv2 = []
            for cp in range(2):
                t_ = wpool.tile([P, 2, C], fp8, name=f"wv{cp}", tag=f"wv{cp}")
                nc.sync.dma_start(t_[:, :, :],
                                  wv_d[:, cp * 1024:(cp + 1) * 1024])
                wv2.append(t_)
            bvf_t = cpool.tile([P, 2 * C], bf16, name="bvf_t")
            nc.sync.dma_start(bvf_t[:], bvf_d[:])
            bexp_t = cpool.tile([P, NT], f32, name="bexp_t")
            nc.sync.dma_start(bexp_t[:], bexp_d[:])
            mask_t = cpool.tile([P, P], bf16, name="mask_t")
            nc.sync.dma_start(mask_t[:], mask_d[:])

            # PE warm-up: junk bf16 matmuls with no DMA dependency to start
            # the HAM clock-gate ramp during the load window.
            warm_src = cpool.tile([P, C + P], bf16, name="warm_src")
            nc.vector.memset(warm_src[:], 0.0)
            ps_warm = pso.tile([P, 512], f32, name="ps_warm", tag="pso")
            for _ in range(10):
                nc.tensor.matmul(ps_warm[:], warm_src[:, C:C + P],
                                 warm_src[:, 0:C], start=True, stop=True)

            # --- Phase 1a: Q^T, K^T in [k, 2, t] fp8 pair layout ---
            # Q^T psum = 32*q_raw; evac: *1/16 + 2*b -> fp8 2q (rms ~0.9).
            # Two 512-col chunks share one [128,1024] psum tile so a single
            # 1024-wide affine evacuates both (half the evac ops, and the
            # PE stays dense enough to hold the HAM clock at full rate).
            # jcp-outer order: the first half of phase 1a touches only
            # X^T[:, 0:1024], hiding the rest of the X^T load.
            qt2 = [qtpool.tile([P, 2, T], fp8, name=f"qt{kp}", tag=f"qt{kp}")
                   for kp in range(2)]
            kt2 = [ktpool.tile([P, 2, T], fp8, name=f"kt{kp}", tag=f"kt{kp}")
                   for kp in range(2)]
            for jcp in range(2):
                jp = slice(jcp * 1024, (jcp + 1) * 1024)
                for kt in range(4):
                    ksl = slice(kt * P, (kt + 1) * P)
                    ps = psq.tile([P, 1024], f32, name="psq1", tag="psq")
                    for h in range(2):
                        js = slice((2 * jcp + h) * 512,
                                   (2 * jcp + h + 1) * 512)
                        for cp in range(2):
                            nc.tensor.matmul(ps[:, h * 512:(h + 1) * 512],
                                             wq2[cp][:, :, ksl],
                                             xt2[cp][:, :, js],
                                             start=(cp == 0), stop=(cp == 1),
                                             perf_mode=DR)
                    nc.scalar.activation(qt2[kt // 2][:, kt % 2, jp], ps[:],
                                         AF.Identity,
                                         bias=bqk_t[:, kt:kt + 1],
                                         scale=1.0 / 16.0)
                    ps2 = psq.tile([P, 1024], f32, name="psq2", tag="psq")
                    for h in range(2):
                        js = slice((2 * jcp + h) * 512,
                                   (2 * jcp + h + 1) * 512)
                        for cp in range(2):
                            nc.tensor.matmul(ps2[:, h * 512:(h + 1) * 512],
                                             wk2[cp][:, :, ksl],
                                             xt2[cp][:, :, js],
                                             start=(cp == 0), stop=(cp == 1),
                                             perf_mode=DR)
                    nc.vector.tensor_scalar(kt2[kt // 2][:, kt % 2, jp],
                                            ps2[:], 1.0 / 16.0,
                                            bqk_t[:, 4 + kt:5 + kt],
                                            ALU.mult, ALU.add)

            # --- Phase 1b: V in natural [t, v] layout, bf16 (true scale) ---
            # Paired the same way: two token tiles per psum/evac.
            v2_sb = []
            for tp in range(8):
                psV = psq.tile([P, 1024], f32, name="psv", tag="psq")
                for h in range(2):
                    tsl = slice((2 * tp + h) * P, (2 * tp + h + 1) * P)
                    for cp in range(2):
                        nc.tensor.matmul(psV[:, h * 512:(h + 1) * 512],
                                         xt2[cp][:, :, tsl],
                                         wv2[cp][:, :, :],
                                         start=(cp == 0), stop=(cp == 1),
                                         perf_mode=DR)
                vt = vpool.tile([P, 2 * C], bf16, name=f"v{tp}", tag=f"v{tp}")
                nc.vector.scalar_tensor_tensor(vt[:], psV[:], 1.0 / 32.0,
                                               bvf_t[:], ALU.mult, ALU.add)
                v2_sb.append(vt)

            # E~ pair tiles [i_lo, s, j]; key row i = 256r + 128s + i_lo.
            et2 = [etpool.tile([P, 2, T], fp8, name=f"et{r}", tag=f"et{r}")
                   for r in range(8)]
            # Zero the garbage diagonal strips read by even-jt read chains:
            # pair r = jt/2 slab 1 is key tile it = jt+1, whose columns
            # [128jt, 128(jt+1)) are fully masked and never written by exp.
            for jt in range(0, NT, 2):
                nc.vector.memset(et2[jt // 2][:, 1, jt * P:(jt + 1) * P], 0.0)
            vp2 = [vppool.tile([P, 2, C], fp8, name=f"vp{r}", tag=f"vp{r}")
                   for r in range(8)]

            # --- Phase 2 per key tile: masked logits + exp + row sums ---
            def p2_it(it):
                isl = slice(it * P, (it + 1) * P)
                parts = []
                w0 = it * P
                # 512-wide psum chunks, 512-aligned in absolute column space
                # (matmul writes must stay within one psum bank).
                for base in range(w0 // 512 * 512, T, 512):
                    lo = max(w0, base)
                    hi = base + 512
                    pslt = psl.tile([P, 512], f32, name="psl", tag="psl")
                    for cp in range(2):
                        nc.tensor.matmul(pslt[:, lo - base:512],
                                         kt2[cp][:, :, isl],
                                         qt2[cp][:, :, lo:hi],
                                         start=(cp == 0), stop=(cp == 1),
                                         perf_mode=DR)
                    if lo == w0:  # triangular mask on the diagonal strip
                        nc.vector.tensor_add(pslt[:, lo - base:lo - base + P],
                                             pslt[:, lo - base:lo - base + P],
                                             mask_t[:])
                    part = spool.tile([P, 1], f32, name="part", tag="part")
                    nc.scalar.activation(et2[it // 2][:, it % 2, lo:hi],
                                         pslt[:, lo - base:512], AF.Exp,
                                         bias=bexp_t[:, it:it + 1],
                                         scale=SEXP, accum_out=part[:])
                    parts.append(part)
                s_t = parts[0]
                for p_ in parts[1:]:
                    ns = spool.tile([P, 1], f32, name="s_t", tag="s_t")
                    nc.vector.tensor_add(ns[:], s_t[:], p_[:])
                    s_t = ns
                r_t = spool.tile([P, 1], f32, name="r_t", tag="r_t")
                nc.vector.reciprocal(r_t[:], s_t[:])
                nc.scalar.activation(
                    vp2[it // 2][:, it % 2, :],
                    v2_sb[it // 2][:, (it % 2) * 512:(it % 2 + 1) * 512],
                    AF.Copy, scale=r_t[:, 0:1])

            # --- Phase 3 per query tile: read[jt] = E~^T pairs @ V~ ---
            ps_o = {}

            def p3_mm(jt, rlo, rhi):
                jsl = slice(jt * P, (jt + 1) * P)
                R = jt // 2 + 1
                if jt not in ps_o:
                    ps_o[jt] = pso.tile([P, 512], f32, name="pso", tag="pso")
                for r in range(rlo, rhi):
                    nc.tensor.matmul(ps_o[jt][:], et2[r][:, :, jsl],
                                     vp2[r][:, :, :],
                                     start=(r == 0), stop=(r == R - 1),
                                     perf_mode=DR)

            def p3_out(jt):
                jsl = slice(jt * P, (jt + 1) * P)
                ost = ospool.tile([P, 512], f32, name="ost", tag="ost")
                nc.vector.tensor_copy(ost[:], ps_o[jt][:])
                # software-DGE queues: keeps the HWDGE path + sync engine
                # free for input loads
                nc.gpsimd.dma_start(out_d[jsl, :], ost[:])

            # Two-iteration lag between p2 producers and p3 consumers so the
            # exp -> reciprocal -> V~-scale chain latency stays hidden
            # behind queued PE work.
            p2_it(0)
            p2_it(1)
            for k in range(2, 14):
                p2_it(k)
                p3_mm(k - 2, 0, (k - 2) // 2 + 1)
                p3_out(k - 2)
            # k=13 done: jt 12,13 fully ready; jt 14/15 all but last pair
            p3_mm(12, 0, 7)
            p3_out(12)
            p3_mm(13, 0, 7)
            p3_out(13)
            p3_mm(14, 0, 7)
            p2_it(14)
            p3_mm(15, 0, 7)
            p2_it(15)
            p3_mm(14, 7, 8)
            p3_out(14)
            p3_mm(15, 7, 8)
            p3_out(15)

    nc.compile()
    return nc


def _get_built():
    global _BUILT
    if _BUILT is None:
        _BUILT = _build_nc()
    return _BUILT


def _make_in_maps(input, Wq, bq, Wk, bk, Wv, bv):
    fp8 = ml_dtypes.float8_e4m3
    bf = ml_dtypes.bfloat16

    input = np.asarray(input, np.float32)
    Wq = np.asarray(Wq, np.float32)
    bq = np.asarray(bq, np.float32)
    Wk = np.asarray(Wk, np.float32)
    bk = np.asarray(bk, np.float32)
    Wv = np.asarray(Wv, np.float32)
    bv = np.asarray(bv, np.float32)

    def pack_w(W):
        # [128, 2048] fp8: [c_lo, cp*1024 + s*512 + k], c = cp*256+s*128+c_lo
        Wt = (W * 32.0).T.reshape(2, 2, P, C)          # [cp, s, c_lo, k]
        return np.ascontiguousarray(
            Wt.transpose(2, 0, 1, 3).reshape(P, 2048)).astype(fp8)

    wq = pack_w(Wq)
    wk = pack_w(Wk)
    wv = pack_w(Wv)

    bqk = np.empty((P, 8), np.float32)
    for kt in range(4):
        bqk[:, kt] = 2.0 * bq[kt * P:(kt + 1) * P]
        bqk[:, 4 + kt] = 2.0 * bk[kt * P:(kt + 1) * P]

    # per-key-row power-of-2 softmax split: t_i = round(log2(2.39 n_i)/2)
    i_idx = np.arange(T)
    n_i = (T - i_idx).astype(np.float64)
    t_i = np.round(0.5 * np.log2(2.39 * n_i)).astype(np.float32)
    bexp = np.ascontiguousarray(
        (-t_i * math.log(2.0)).reshape(NT, P).T).astype(np.float32)

    pp = np.arange(P)[:, None]
    xx = np.arange(P)[None, :]
    mask = np.where(xx < pp, NEG, 0.0).astype(bf)

    bvf = np.ascontiguousarray(
        np.broadcast_to(np.tile(bv, 2).astype(bf), (P, 2 * C)))

    in_maps = []
    for b in range(B):
        in_maps.append({
            "xt": np.ascontiguousarray(input[b].T).astype(fp8),
            "wq": wq, "wk": wk, "wv": wv,
            "bqk": bqk, "bexp": bexp, "mask": mask, "bvf": bvf,
        })
    return in_maps


def kernel(input, Wq, bq, Wk, bk, Wv, bv, _trace=False):
    from concourse.bass_utils import run_bass_kernel_spmd

    nc = _get_built()
    in_maps = _make_in_maps(input, Wq, bq, Wk, bk, Wv, bv)
    res = run_bass_kernel_spmd(nc, in_maps, core_ids=list(range(NCORES)),
                               trace=_trace)
    input = np.asarray(input, np.float32)
    out = np.empty((B, T, 2 * C), np.float32)
    for b in range(B):
        out[b, :, 0:C] = input[b]
        out[b, :, C:] = res.results[b]["out"]
    if _trace:
        kernel.last_result = res
    return out


# revision 21
# speedup vs baseline: 1.1659x; 1.0805x over previous
"""Trainium2 Bass kernel for an attention block with a non-standard
(query-axis) softmax and causal mask.  fp8 DoubleRow version.

Math per batch element b (T=2048 tokens, C=K=V=512):
    q = x @ Wq.T + bq ; k = x @ Wk.T + bk ; v = x @ Wv.T + bv
    logits[j, i] = q[j] . k[i]                     (j=query, i=key)
    masked = -inf where i > j
    probs = softmax(masked / sqrt(512), axis=j)    <-- softmax over QUERY axis
    read[j] = sum_i probs[j, i] * v[i]
    out = concat(x, read)                          [T, 1024]

Distribution: pure data-parallel, batch b -> core b (8 batches, 8 cores),
weights replicated, no collectives.  The passthrough half of the output is
assembled on the host (np.concatenate); the device computes only read.

All matmuls run in fp8(e4m3) with perf_mode=DoubleRow: one instruction
contracts TWO 128-row slabs (lhsT [128,2,M], rhs [128,2,N]) at the same
rate a bf16 matmul contracts one -- 2x effective PE throughput (measured
222ns per [128out x 512free x 256contract] on HW).

Scale management (e4m3: max 240, min normal 2^-6):
  - X cast to fp8 directly (rms 1).  Weights scaled by 32 (rms 0.64); the
    1/32 is folded into the PSUM-evacuation affines.
  - Q,K stored as fp8 2q, 2k (rms 0.9); the extra 2*2 and the softmax
    1/sqrt(512) fold into the ACT exp scale.
  - The softmax normalizer 1/sum_j e spans [1/2048, 1] across key rows i.
    That 11-octave range is split evenly between the two read-matmul
    operands: E~[i,j] = e[i,j]*2^-t_i (via a static per-partition bias
    -t_i*ln2 added inside the exp) and V~[i,v] = v[i,v] / sum_j E~[i,j].
    Then E~ . V~ == probs . v exactly, and both operands sit near the
    middle of the fp8 range (t_i = round(log2(2.39*n_i)/2), n_i = 2048-i).

Engine budget (PSUM can only be read by ACT/DVE; GPSIMD is SBUF-only):
  PE ~56us (248 DR matmuls + warmups), ACT ~35us (Q/K affines via
  Identity+bias, 24 exps with accum), DVE ~39us (K affines, V bias-stt,
  triangular mask adds on the 128-wide diagonal strips, reciprocals,
  PSUM->SBUF output copies), GPSIMD ~11us (V~ scaling into fp8, part sums,
  zero strips for the even-jt diagonal pairs, nothing touching PSUM).

Phase 2 (logits+exp) and phase 3 (read) are emission-interleaved so the
in-order PE queue never waits long on the ACT exp pipeline; the last two
read rows' accumulation chains are split so only one pair of matmuls
remains after the final exp.
"""

import math

import numpy as np
import ml_dtypes

P = 128
B, T, C = 8, 2048, 512
NT = T // P     # 16 row tiles
NCORES = 8
NEG = -1e30
SEXP = 1.0 / (4.0 * math.sqrt(512.0))

_BUILT = None


def _build_nc():
    import concourse.mybir as mybir
    import concourse.tile as tile
    from concourse import bacc

    f32 = mybir.dt.float32
    bf16 = mybir.dt.bfloat16
    fp8 = mybir.dt.float8e4
    AF = mybir.ActivationFunctionType
    ALU = mybir.AluOpType
    DR = mybir.MatmulPerfMode.DoubleRow

    nc = bacc.Bacc("TRN2", target_bir_lowering=False, debug=False,
                   num_devices=NCORES)

    xt_d = nc.dram_tensor("xt", [C, T], fp8, kind="ExternalInput")
    wq_d = nc.dram_tensor("wq", [P, 2048], fp8, kind="ExternalInput")
    wk_d = nc.dram_tensor("wk", [P, 2048], fp8, kind="ExternalInput")
    wv_d = nc.dram_tensor("wv", [P, 2048], fp8, kind="ExternalInput")
    bqk_d = nc.dram_tensor("bqk", [P, 8], f32, kind="ExternalInput")
    bexp_d = nc.dram_tensor("bexp", [P, NT], f32, kind="ExternalInput")
    mask_d = nc.dram_tensor("mask", [P, P], bf16, kind="ExternalInput")
    bvf_d = nc.dram_tensor("bvf", [P, 2 * C], bf16, kind="ExternalInput")
    out_d = nc.dram_tensor("out", [T, C], f32, kind="ExternalOutput")

    with tile.TileContext(nc) as tc:
        with (
            tc.tile_pool(name="const", bufs=1) as cpool,
            tc.tile_pool(name="w", bufs=1) as wpool,
            tc.tile_pool(name="xt", bufs=1) as xtpool,
            tc.tile_pool(name="qt", bufs=1) as qtpool,
            tc.tile_pool(name="kt", bufs=1) as ktpool,
            tc.tile_pool(name="v", bufs=1) as vpool,
            tc.tile_pool(name="vp", bufs=1) as vppool,
            tc.tile_pool(name="et", bufs=1) as etpool,
            tc.tile_pool(name="small", bufs=16) as spool,
            tc.tile_pool(name="ostage", bufs=3) as ospool,
            tc.tile_pool(name="psq", bufs=2, space="PSUM") as psq,   # 2x1024
            tc.tile_pool(name="psl", bufs=2, space="PSUM") as psl,   # 2x1024
        ):
            # --- loads ---
            # dma_start issue costs ~0.6us of engine time each, so the
            # issues are split across both HWDGE engines (sync + scalar)
            # and ordered most-urgent-first on each.
            xt2 = [xtpool.tile([P, 2, T], fp8, name=f"xt{cp}", tag=f"xt{cp}")
                   for cp in range(2)]
            for cp in range(2):
                for s in range(2):
                    r0 = (2 * cp + s) * P
                    nc.sync.dma_start(xt2[cp][:, s, :], xt_d[r0:r0 + P, :])
            wq2 = []
            for cp in range(2):
                t_ = wpool.tile([P, 2, C], fp8, name=f"wq{cp}", tag=f"wq{cp}")
                nc.scalar.dma_start(t_[:, :, :],
                                    wq_d[:, cp * 1024:(cp + 1) * 1024])
                wq2.append(t_)
            wk2 = []
            for cp in range(2):
                t_ = wpool.tile([P, 2, C], fp8, name=f"wk{cp}", tag=f"wk{cp}")
                nc.scalar.dma_start(t_[:, :, :],
                                    wk_d[:, cp * 1024:(cp + 1) * 1024])
                wk2.append(t_)
            bqk_t = cpool.tile([P, 8], f32, name="bqk_t")
            nc.scalar.dma_start(bqk_t[:], bqk_d[:])
            wv2 = []
            for cp in range(2):
                t_ = wpool.tile([P, 2, C], fp8, name=f"wv{cp}", tag=f"wv{cp}")
                nc.sync.dma_start(t_[:, :, :],
                                  wv_d[:, cp * 1024:(cp + 1) * 1024])
                wv2.append(t_)
            bvf_t = cpool.tile([P, 2 * C], bf16, name="bvf_t")
            nc.sync.dma_start(bvf_t[:], bvf_d[:])
            bexp_t = cpool.tile([P, NT], f32, name="bexp_t")
            nc.sync.dma_start(bexp_t[:], bexp_d[:])
            mask_t = cpool.tile([P, P], bf16, name="mask_t")
            nc.sync.dma_start(mask_t[:], mask_d[:])

            # PE warm-up: junk bf16 matmuls with no DMA dependency to start
            # the HAM clock-gate ramp during the load window.
            warm_src = cpool.tile([P, C + P], bf16, name="warm_src")
            nc.vector.memset(warm_src[:], 0.0)
            ps_warm = psq.tile([P, 1024], f32, name="ps_warm", tag="psq")
            for _ in range(10):
                nc.tensor.matmul(ps_warm[:, 0:512], warm_src[:, C:C + P],
                                 warm_src[:, 0:C], start=True, stop=True)

            # --- Phase 1a: Q^T, K^T in [k, 2, t] fp8 pair layout ---
            # Q^T psum = 32*q_raw; evac: *1/16 + 2*b -> fp8 2q (rms ~0.9).
            # Two 512-col chunks share one [128,1024] psum tile so a single
            # 1024-wide affine evacuates both (half the evac ops, and the
            # PE stays dense enough to hold the HAM clock at full rate).
            # jcp-outer order: the first half of phase 1a touches only
            # X^T[:, 0:1024], hiding the rest of the X^T load.
            qt2 = [qtpool.tile([P, 2, T], fp8, name=f"qt{kp}", tag=f"qt{kp}")
                   for kp in range(2)]
            kt2 = [ktpool.tile([P, 2, T], fp8, name=f"kt{kp}", tag=f"kt{kp}")
                   for kp in range(2)]
            for jcp in range(2):
                jp = slice(jcp * 1024, (jcp + 1) * 1024)
                for kt in range(4):
                    ksl = slice(kt * P, (kt + 1) * P)
                    ps = psq.tile([P, 1024], f32, name="psq1", tag="psq")
                    for h in range(2):
                        js = slice((2 * jcp + h) * 512,
                                   (2 * jcp + h + 1) * 512)
                        for cp in range(2):
                            nc.tensor.matmul(ps[:, h * 512:(h + 1) * 512],
                                             wq2[cp][:, :, ksl],
                                             xt2[cp][:, :, js],
                                             start=(cp == 0), stop=(cp == 1),
                                             perf_mode=DR)
                    nc.scalar.activation(qt2[kt // 2][:, kt % 2, jp], ps[:],
                                         AF.Identity,
                                         bias=bqk_t[:, kt:kt + 1],
                                         scale=1.0 / 16.0)
                    ps2 = psq.tile([P, 1024], f32, name="psq2", tag="psq")
                    for h in range(2):
                        js = slice((2 * jcp + h) * 512,
                                   (2 * jcp + h + 1) * 512)
                        for cp in range(2):
                            nc.tensor.matmul(ps2[:, h * 512:(h + 1) * 512],
                                             wk2[cp][:, :, ksl],
                                             xt2[cp][:, :, js],
                                             start=(cp == 0), stop=(cp == 1),
                                             perf_mode=DR)
                    nc.vector.tensor_scalar(kt2[kt // 2][:, kt % 2, jp],
                                            ps2[:], 1.0 / 16.0,
                                            bqk_t[:, 4 + kt:5 + kt],
                                            ALU.mult, ALU.add)

            # --- Phase 1b: V in natural [t, v] layout, bf16 (true scale) ---
            # Paired (two token tiles per psum/evac) and emitted interleaved
            # into the phase-2 loop so the DVE evacuations don't bunch up in
            # front of phase 2's mask adds.
            v2_sb = [vpool.tile([P, 2 * C], bf16, name=f"v{tp}", tag=f"v{tp}")
                     for tp in range(8)]

            def p1b_pair(tp):
                psV = psq.tile([P, 1024], f32, name="psv", tag="psq")
                for h in range(2):
                    tsl = slice((2 * tp + h) * P, (2 * tp + h + 1) * P)
                    for cp in range(2):
                        nc.tensor.matmul(psV[:, h * 512:(h + 1) * 512],
                                         xt2[cp][:, :, tsl],
                                         wv2[cp][:, :, :],
                                         start=(cp == 0), stop=(cp == 1),
                                         perf_mode=DR)
                nc.vector.scalar_tensor_tensor(v2_sb[tp][:], psV[:],
                                               1.0 / 32.0, bvf_t[:],
                                               ALU.mult, ALU.add)

            # E~ pair tiles [i_lo, s, j]; key row i = 256r + 128s + i_lo.
            et2 = [etpool.tile([P, 2, T], fp8, name=f"et{r}", tag=f"et{r}")
                   for r in range(8)]
            # Zero the garbage diagonal strips read by even-jt read chains:
            # pair r = jt/2 slab 1 is key tile it = jt+1, whose columns
            # [128jt, 128(jt+1)) are fully masked and never written by exp.
            for jt in range(0, NT, 2):
                nc.vector.memset(et2[jt // 2][:, 1, jt * P:(jt + 1) * P], 0.0)
            vp2 = [vppool.tile([P, 2, C], fp8, name=f"vp{r}", tag=f"vp{r}")
                   for r in range(8)]

            # --- Phase 2 per key tile: masked logits + exp + row sums ---
            def p2_it(it):
                isl = slice(it * P, (it + 1) * P)
                parts = []
                w0 = it * P
                # 1024-wide psum chunks, 1024-aligned in absolute column
                # space; each 512-aligned matmul subchunk stays in one bank.
                for base in range(w0 // 1024 * 1024, T, 1024):
                    lo = max(w0, base)
                    hi = base + 1024
                    pslt = psl.tile([P, 1024], f32, name="psl", tag="psl")
                    s0 = lo
                    while s0 < hi:
                        s1 = min((s0 // 512 + 1) * 512, hi)
                        for cp in range(2):
                            nc.tensor.matmul(pslt[:, s0 - base:s1 - base],
                                             kt2[cp][:, :, isl],
                                             qt2[cp][:, :, s0:s1],
                                             start=(cp == 0), stop=(cp == 1),
                                             perf_mode=DR)
                        s0 = s1
                    if lo == w0:  # triangular mask on the diagonal strip
                        nc.vector.tensor_add(pslt[:, lo - base:lo - base + P],
                                             pslt[:, lo - base:lo - base + P],
                                             mask_t[:])
                    part = spool.tile([P, 1], f32, name="part", tag="part")
                    nc.scalar.activation(et2[it // 2][:, it % 2, lo:hi],
                                         pslt[:, lo - base:1024], AF.Exp,
                                         bias=bexp_t[:, it:it + 1],
                                         scale=SEXP, accum_out=part[:])
                    parts.append(part)
                s_t = parts[0]
                for p_ in parts[1:]:
                    ns = spool.tile([P, 1], f32, name="s_t", tag="s_t")
                    nc.vector.tensor_add(ns[:], s_t[:], p_[:])
                    s_t = ns
                r_t = spool.tile([P, 1], f32, name="r_t", tag="r_t")
                nc.vector.reciprocal(r_t[:], s_t[:])
                nc.scalar.activation(
                    vp2[it // 2][:, it % 2, :],
                    v2_sb[it // 2][:, (it % 2) * 512:(it % 2 + 1) * 512],
                    AF.Copy, scale=r_t[:, 0:1])

            # --- Phase 3: read rows, two query tiles (jt=2m, 2m+1) share
            # one [128,1024] psum tile and one paired evacuation.  Both
            # column chains accumulate over the same r = 0..m pair range.
            ps_g = {}

            def p3_group(m, rlo, rhi):
                if m not in ps_g:
                    ps_g[m] = psq.tile([P, 1024], f32, name="psg", tag="psq")
                for r in range(rlo, rhi):
                    for h in range(2):
                        jsl = slice((2 * m + h) * P, (2 * m + h + 1) * P)
                        nc.tensor.matmul(ps_g[m][:, h * 512:(h + 1) * 512],
                                         et2[r][:, :, jsl], vp2[r][:, :, :],
                                         start=(r == 0), stop=(r == m),
                                         perf_mode=DR)

            def p3_out(m):
                ost = ospool.tile([P, 1024], f32, name="ost", tag="ost")
                nc.vector.tensor_copy(ost[:], ps_g[m][:])
                # software-DGE queues: keeps the HWDGE path free for loads
                r0 = 2 * m * P
                nc.gpsimd.dma_start(out_d[r0:r0 + P, :], ost[:, 0:512])
                nc.gpsimd.dma_start(out_d[r0 + P:r0 + 2 * P, :],
                                    ost[:, 512:1024])

            for k in range(14):
                if k % 2 == 0:
                    p1b_pair(k // 2)
                p2_it(k)
                if k >= 3 and k % 2 == 1:
                    m = (k - 3) // 2
                    p3_group(m, 0, m + 1)
                    p3_out(m)
            # k=13 done: groups 6 (jt 12,13) fully ready; 7 all but r=7
            p3_group(6, 0, 7)
            p3_out(6)
            p3_group(7, 0, 7)
            p1b_pair(7)
            p2_it(14)
            p2_it(15)
            p3_group(7, 7, 8)
            p3_out(7)

    nc.compile()
    return nc


def _get_built():
    global _BUILT
    if _BUILT is None:
        _BUILT = _build_nc()
    return _BUILT


def _make_in_maps(input, Wq, bq, Wk, bk, Wv, bv):
    fp8 = ml_dtypes.float8_e4m3
    bf = ml_dtypes.bfloat16

    input = np.asarray(input, np.float32)
    Wq = np.asarray(Wq, np.float32)
    bq = np.asarray(bq, np.float32)
    Wk = np.asarray(Wk, np.float32)
    bk = np.asarray(bk, np.float32)
    Wv = np.asarray(Wv, np.float32)
    bv = np.asarray(bv, np.float32)

    def pack_w(W):
        # [128, 2048] fp8: [c_lo, cp*1024 + s*512 + k], c = cp*256+s*128+c_lo
        Wt = (W * 32.0).T.reshape(2, 2, P, C)          # [cp, s, c_lo, k]
        return np.ascontiguousarray(
            Wt.transpose(2, 0, 1, 3).reshape(P, 2048)).astype(fp8)

    wq = pack_w(Wq)
    wk = pack_w(Wk)
    wv = pack_w(Wv)

    bqk = np.empty((P, 8), np.float32)
    for kt in range(4):
        bqk[:, kt] = 2.0 * bq[kt * P:(kt + 1) * P]
        bqk[:, 4 + kt] = 2.0 * bk[kt * P:(kt + 1) * P]

    # per-key-row power-of-2 softmax split: t_i = round(log2(2.39 n_i)/2)
    i_idx = np.arange(T)
    n_i = (T - i_idx).astype(np.float64)
    t_i = np.round(0.5 * np.log2(2.39 * n_i)).astype(np.float32)
    bexp = np.ascontiguousarray(
        (-t_i * math.log(2.0)).reshape(NT, P).T).astype(np.float32)

    pp = np.arange(P)[:, None]
    xx = np.arange(P)[None, :]
    mask = np.where(xx < pp, NEG, 0.0).astype(bf)

    bvf = np.ascontiguousarray(
        np.broadcast_to(np.tile(bv, 2).astype(bf), (P, 2 * C)))

    in_maps = []
    for b in range(B):
        in_maps.append({
            "xt": np.ascontiguousarray(input[b].T).astype(fp8),
            "wq": wq, "wk": wk, "wv": wv,
            "bqk": bqk, "bexp": bexp, "mask": mask, "bvf": bvf,
        })
    return in_maps


def kernel(input, Wq, bq, Wk, bk, Wv, bv, _trace=False):
    from concourse.bass_utils import run_bass_kernel_spmd

    nc = _get_built()
    in_maps = _make_in_maps(input, Wq, bq, Wk, bk, Wv, bv)
    res = run_bass_kernel_spmd(nc, in_maps, core_ids=list(range(NCORES)),
                               trace=_trace)
    input = np.asarray(input, np.float32)
    out = np.empty((B, T, 2 * C), np.float32)
    for b in range(B):
        out[b, :, 0:C] = input[b]
        out[b, :, C:] = res.results[b]["out"]
    if _trace:
        kernel.last_result = res
    return out


# revision 23
# speedup vs baseline: 1.1744x; 1.0073x over previous
"""Trainium2 Bass kernel for an attention block with a non-standard
(query-axis) softmax and causal mask.  fp8 DoubleRow version.

Math per batch element b (T=2048 tokens, C=K=V=512):
    q = x @ Wq.T + bq ; k = x @ Wk.T + bk ; v = x @ Wv.T + bv
    logits[j, i] = q[j] . k[i]                     (j=query, i=key)
    masked = -inf where i > j
    probs = softmax(masked / sqrt(512), axis=j)    <-- softmax over QUERY axis
    read[j] = sum_i probs[j, i] * v[i]
    out = concat(x, read)                          [T, 1024]

Distribution: pure data-parallel, batch b -> core b (8 batches, 8 cores),
weights replicated, no collectives.  The passthrough half of the output is
assembled on the host (np.concatenate); the device computes only read.

All matmuls run in fp8(e4m3) with perf_mode=DoubleRow: one instruction
contracts TWO 128-row slabs (lhsT [128,2,M], rhs [128,2,N]) at the same
rate a bf16 matmul contracts one -- 2x effective PE throughput (measured
222ns per [128out x 512free x 256contract] on HW).

Scale management (e4m3: max 240, min normal 2^-6):
  - X cast to fp8 directly (rms 1).  Weights scaled by 32 (rms 0.64); the
    1/32 is folded into the PSUM-evacuation affines.
  - Q,K stored as fp8 2q, 2k (rms 0.9); the extra 2*2 and the softmax
    1/sqrt(512) fold into the ACT exp scale.
  - The softmax normalizer 1/sum_j e spans [1/2048, 1] across key rows i.
    That 11-octave range is split evenly between the two read-matmul
    operands: E~[i,j] = e[i,j]*2^-t_i (via a static per-partition bias
    -t_i*ln2 added inside the exp) and V~[i,v] = v[i,v] / sum_j E~[i,j].
    Then E~ . V~ == probs . v exactly, and both operands sit near the
    middle of the fp8 range (t_i = round(log2(2.39*n_i)/2), n_i = 2048-i).

Engine budget (PSUM can only be read by ACT/DVE; GPSIMD is SBUF-only):
  PE ~56us (248 DR matmuls + warmups), ACT ~35us (Q/K affines via
  Identity+bias, 24 exps with accum), DVE ~39us (K affines, V bias-stt,
  triangular mask adds on the 128-wide diagonal strips, reciprocals,
  PSUM->SBUF output copies), GPSIMD ~11us (V~ scaling into fp8, part sums,
  zero strips for the even-jt diagonal pairs, nothing touching PSUM).

Phase 2 (logits+exp) and phase 3 (read) are emission-interleaved so the
in-order PE queue never waits long on the ACT exp pipeline; the last two
read rows' accumulation chains are split so only one pair of matmuls
remains after the final exp.
"""

import math

import numpy as np
import ml_dtypes

P = 128
B, T, C = 8, 2048, 512
NT = T // P     # 16 row tiles
NCORES = 8
NEG = -1e30
SEXP = 1.0 / (4.0 * math.sqrt(512.0))

_BUILT = None


def _build_nc():
    import concourse.mybir as mybir
    import concourse.tile as tile
    from concourse import bacc

    f32 = mybir.dt.float32
    bf16 = mybir.dt.bfloat16
    fp8 = mybir.dt.float8e4
    AF = mybir.ActivationFunctionType
    ALU = mybir.AluOpType
    DR = mybir.MatmulPerfMode.DoubleRow

    nc = bacc.Bacc("TRN2", target_bir_lowering=False, debug=False,
                   num_devices=NCORES)

    xt_d = nc.dram_tensor("xt", [C, T], fp8, kind="ExternalInput")
    wq_d = nc.dram_tensor("wq", [P, 2048], fp8, kind="ExternalInput")
    wk_d = nc.dram_tensor("wk", [P, 2048], fp8, kind="ExternalInput")
    wv_d = nc.dram_tensor("wv", [P, 2048], fp8, kind="ExternalInput")
    bqk_d = nc.dram_tensor("bqk", [P, 8], f32, kind="ExternalInput")
    bexp_d = nc.dram_tensor("bexp", [P, NT], f32, kind="ExternalInput")
    mask_d = nc.dram_tensor("mask", [P, P], bf16, kind="ExternalInput")
    bvf_d = nc.dram_tensor("bvf", [P, 2 * C], bf16, kind="ExternalInput")
    out_d = nc.dram_tensor("out", [T, C], f32, kind="ExternalOutput")

    with tile.TileContext(nc) as tc:
        with (
            tc.tile_pool(name="const", bufs=1) as cpool,
            tc.tile_pool(name="w", bufs=1) as wpool,
            tc.tile_pool(name="xt", bufs=1) as xtpool,
            tc.tile_pool(name="qt", bufs=1) as qtpool,
            tc.tile_pool(name="kt", bufs=1) as ktpool,
            tc.tile_pool(name="v", bufs=1) as vpool,
            tc.tile_pool(name="vp", bufs=1) as vppool,
            tc.tile_pool(name="et", bufs=1) as etpool,
            tc.tile_pool(name="small", bufs=16) as spool,
            tc.tile_pool(name="ostage", bufs=3) as ospool,
            tc.tile_pool(name="psq", bufs=2, space="PSUM") as psq,   # 2x1024
            tc.tile_pool(name="psl", bufs=2, space="PSUM") as psl,   # 2x1024
        ):
            # --- loads ---
            # dma_start issue costs ~0.6us of engine time each, so the
            # issues are split across both HWDGE engines (sync + scalar)
            # and ordered most-urgent-first on each.
            xt2 = [xtpool.tile([P, 2, T], fp8, name=f"xt{cp}", tag=f"xt{cp}")
                   for cp in range(2)]
            for cp in range(2):
                for s in range(2):
                    r0 = (2 * cp + s) * P
                    nc.sync.dma_start(xt2[cp][:, s, :], xt_d[r0:r0 + P, :])
            wq2 = []
            for cp in range(2):
                t_ = wpool.tile([P, 2, C], fp8, name=f"wq{cp}", tag=f"wq{cp}")
                nc.scalar.dma_start(t_[:, :, :],
                                    wq_d[:, cp * 1024:(cp + 1) * 1024])
                wq2.append(t_)
            wk2 = []
            for cp in range(2):
                t_ = wpool.tile([P, 2, C], fp8, name=f"wk{cp}", tag=f"wk{cp}")
                nc.scalar.dma_start(t_[:, :, :],
                                    wk_d[:, cp * 1024:(cp + 1) * 1024])
                wk2.append(t_)
            bqk_t = cpool.tile([P, 8], f32, name="bqk_t")
            nc.scalar.dma_start(bqk_t[:], bqk_d[:])
            wv2 = []
            for cp in range(2):
                t_ = wpool.tile([P, 2, C], fp8, name=f"wv{cp}", tag=f"wv{cp}")
                nc.sync.dma_start(t_[:, :, :],
                                  wv_d[:, cp * 1024:(cp + 1) * 1024])
                wv2.append(t_)
            bvf_t = cpool.tile([P, 2 * C], bf16, name="bvf_t")
            nc.sync.dma_start(bvf_t[:], bvf_d[:])
            bexp_t = cpool.tile([P, NT], f32, name="bexp_t")
            nc.sync.dma_start(bexp_t[:], bexp_d[:])
            mask_t = cpool.tile([P, P], bf16, name="mask_t")
            nc.sync.dma_start(mask_t[:], mask_d[:])

            # PE warm-up: junk bf16 matmuls with no DMA dependency to start
            # the HAM clock-gate ramp during the load window.
            warm_src = cpool.tile([P, C + P], bf16, name="warm_src")
            nc.vector.memset(warm_src[:], 0.0)
            ps_warm = psq.tile([P, 1024], f32, name="ps_warm", tag="psq")
            for _ in range(7):
                nc.tensor.matmul(ps_warm[:, 0:512], warm_src[:, C:C + P],
                                 warm_src[:, 0:C], start=True, stop=True)

            # --- Phase 1a: Q^T, K^T in [k, 2, t] fp8 pair layout ---
            # Q^T psum = 32*q_raw; evac: *1/16 + 2*b -> fp8 2q (rms ~0.9).
            # Two 512-col chunks share one [128,1024] psum tile so a single
            # 1024-wide affine evacuates both (half the evac ops, and the
            # PE stays dense enough to hold the HAM clock at full rate).
            # jcp-outer order: the first half of phase 1a touches only
            # X^T[:, 0:1024], hiding the rest of the X^T load.
            qt2 = [qtpool.tile([P, 2, T], fp8, name=f"qt{kp}", tag=f"qt{kp}")
                   for kp in range(2)]
            kt2 = [ktpool.tile([P, 2, T], fp8, name=f"kt{kp}", tag=f"kt{kp}")
                   for kp in range(2)]
            for jcp in range(2):
                jp = slice(jcp * 1024, (jcp + 1) * 1024)
                for kt in range(4):
                    ksl = slice(kt * P, (kt + 1) * P)
                    ps = psq.tile([P, 1024], f32, name="psq1", tag="psq")
                    for h in range(2):
                        js = slice((2 * jcp + h) * 512,
                                   (2 * jcp + h + 1) * 512)
                        for cp in range(2):
                            nc.tensor.matmul(ps[:, h * 512:(h + 1) * 512],
                                             wq2[cp][:, :, ksl],
                                             xt2[cp][:, :, js],
                                             start=(cp == 0), stop=(cp == 1),
                                             perf_mode=DR)
                    nc.scalar.activation(qt2[kt // 2][:, kt % 2, jp], ps[:],
                                         AF.Identity,
                                         bias=bqk_t[:, kt:kt + 1],
                                         scale=1.0 / 16.0)
                    ps2 = psq.tile([P, 1024], f32, name="psq2", tag="psq")
                    for h in range(2):
                        js = slice((2 * jcp + h) * 512,
                                   (2 * jcp + h + 1) * 512)
                        for cp in range(2):
                            nc.tensor.matmul(ps2[:, h * 512:(h + 1) * 512],
                                             wk2[cp][:, :, ksl],
                                             xt2[cp][:, :, js],
                                             start=(cp == 0), stop=(cp == 1),
                                             perf_mode=DR)
                    nc.vector.tensor_scalar(kt2[kt // 2][:, kt % 2, jp],
                                            ps2[:], 1.0 / 16.0,
                                            bqk_t[:, 4 + kt:5 + kt],
                                            ALU.mult, ALU.add)

            # --- Phase 1b: V in natural [t, v] layout, bf16 (true scale) ---
            # Paired (two token tiles per psum/evac) and emitted interleaved
            # into the phase-2 loop so the DVE evacuations don't bunch up in
            # front of phase 2's mask adds.
            v2_sb = [vpool.tile([P, 2 * C], bf16, name=f"v{tp}", tag=f"v{tp}")
                     for tp in range(8)]

            def p1b_pair(tp):
                psV = psq.tile([P, 1024], f32, name="psv", tag="psq")
                for h in range(2):
                    tsl = slice((2 * tp + h) * P, (2 * tp + h + 1) * P)
                    for cp in range(2):
                        nc.tensor.matmul(psV[:, h * 512:(h + 1) * 512],
                                         xt2[cp][:, :, tsl],
                                         wv2[cp][:, :, :],
                                         start=(cp == 0), stop=(cp == 1),
                                         perf_mode=DR)
                nc.vector.scalar_tensor_tensor(v2_sb[tp][:], psV[:],
                                               1.0 / 32.0, bvf_t[:],
                                               ALU.mult, ALU.add)

            # E~ pair tiles [i_lo, s, j]; key row i = 256r + 128s + i_lo.
            et2 = [etpool.tile([P, 2, T], fp8, name=f"et{r}", tag=f"et{r}")
                   for r in range(8)]
            # Zero the garbage diagonal strips read by even-jt read chains:
            # pair r = jt/2 slab 1 is key tile it = jt+1, whose columns
            # [128jt, 128(jt+1)) are fully masked and never written by exp.
            for jt in range(0, NT, 2):
                nc.vector.memset(et2[jt // 2][:, 1, jt * P:(jt + 1) * P], 0.0)
            vp2 = [vppool.tile([P, 2, C], fp8, name=f"vp{r}", tag=f"vp{r}")
                   for r in range(8)]

            # --- Phase 2 per key tile: masked logits + exp + row sums ---
            def p2_it(it):
                isl = slice(it * P, (it + 1) * P)
                parts = []
                w0 = it * P
                # 1024-wide psum chunks, 1024-aligned in absolute column
                # space; each 512-aligned matmul subchunk stays in one bank.
                for base in range(w0 // 1024 * 1024, T, 1024):
                    lo = max(w0, base)
                    hi = base + 1024
                    pslt = psl.tile([P, 1024], f32, name="psl", tag="psl")
                    s0 = lo
                    while s0 < hi:
                        s1 = min((s0 // 512 + 1) * 512, hi)
                        for cp in range(2):
                            nc.tensor.matmul(pslt[:, s0 - base:s1 - base],
                                             kt2[cp][:, :, isl],
                                             qt2[cp][:, :, s0:s1],
                                             start=(cp == 0), stop=(cp == 1),
                                             perf_mode=DR)
                        s0 = s1
                    if lo == w0:  # triangular mask on the diagonal strip
                        nc.vector.tensor_add(pslt[:, lo - base:lo - base + P],
                                             pslt[:, lo - base:lo - base + P],
                                             mask_t[:])
                    part = spool.tile([P, 1], f32, name="part", tag="part")
                    nc.scalar.activation(et2[it // 2][:, it % 2, lo:hi],
                                         pslt[:, lo - base:1024], AF.Exp,
                                         bias=bexp_t[:, it:it + 1],
                                         scale=SEXP, accum_out=part[:])
                    parts.append(part)
                s_t = parts[0]
                for p_ in parts[1:]:
                    ns = spool.tile([P, 1], f32, name="s_t", tag="s_t")
                    nc.vector.tensor_add(ns[:], s_t[:], p_[:])
                    s_t = ns
                r_t = spool.tile([P, 1], f32, name="r_t", tag="r_t")
                nc.vector.reciprocal(r_t[:], s_t[:])
                nc.scalar.activation(
                    vp2[it // 2][:, it % 2, :],
                    v2_sb[it // 2][:, (it % 2) * 512:(it % 2 + 1) * 512],
                    AF.Copy, scale=r_t[:, 0:1])

            # --- Phase 3: read rows, two query tiles (jt=2m, 2m+1) share
            # one [128,1024] psum tile and one paired evacuation.  Both
            # column chains accumulate over the same r = 0..m pair range.
            ps_g = {}

            def p3_group(m, rlo, rhi):
                if m not in ps_g:
                    ps_g[m] = psq.tile([P, 1024], f32, name="psg", tag="psq")
                for r in range(rlo, rhi):
                    for h in range(2):
                        jsl = slice((2 * m + h) * P, (2 * m + h + 1) * P)
                        nc.tensor.matmul(ps_g[m][:, h * 512:(h + 1) * 512],
                                         et2[r][:, :, jsl], vp2[r][:, :, :],
                                         start=(r == 0), stop=(r == m),
                                         perf_mode=DR)

            def p3_out(m, half=None):
                # all loads are done by now, so the idle sync HWDGE path
                # carries the output (gpsimd's DGE drain is expensive)
                ost = ospool.tile([P, 1024], f32, name="ost", tag="ost")
                r0 = 2 * m * P
                if half is None:
                    nc.vector.tensor_copy(ost[:], ps_g[m][:])
                    nc.sync.dma_start(out_d[r0:r0 + P, :], ost[:, 0:512])
                    nc.sync.dma_start(out_d[r0 + P:r0 + 2 * P, :],
                                      ost[:, 512:1024])
                else:
                    h = half
                    nc.vector.tensor_copy(ost[:, h * 512:(h + 1) * 512],
                                          ps_g[m][:, h * 512:(h + 1) * 512])
                    nc.sync.dma_start(out_d[r0 + h * P:r0 + (h + 1) * P, :],
                                      ost[:, h * 512:(h + 1) * 512])

            p1b_pair(0)
            p1b_pair(1)
            for k in range(14):
                if k % 2 == 0 and k // 2 + 2 < 8:
                    p1b_pair(k // 2 + 2)
                p2_it(k)
                if k >= 3 and k % 2 == 1:
                    m = (k - 3) // 2
                    p3_group(m, 0, m + 1)
                    p3_out(m)
            # k=13 done: group 6 (jt 12,13) fully ready; 7 all but r=7
            p3_group(6, 0, 7)
            p3_out(6)
            p3_group(7, 0, 7)
            p2_it(14)
            p2_it(15)
            # final pair: evacuate each half right behind its last matmul
            jsl = slice(14 * P, 15 * P)
            nc.tensor.matmul(ps_g[7][:, 0:512], et2[7][:, :, jsl],
                             vp2[7][:, :, :], start=False, stop=True,
                             perf_mode=DR)
            p3_out(7, half=0)
            jsl = slice(15 * P, 16 * P)
            nc.tensor.matmul(ps_g[7][:, 512:1024], et2[7][:, :, jsl],
                             vp2[7][:, :, :], start=False, stop=True,
                             perf_mode=DR)
            p3_out(7, half=1)

    nc.compile()
    return nc


def _get_built():
    global _BUILT
    if _BUILT is None:
        _BUILT = _build_nc()
    return _BUILT


def _make_in_maps(input, Wq, bq, Wk, bk, Wv, bv):
    fp8 = ml_dtypes.float8_e4m3
    bf = ml_dtypes.bfloat16

    input = np.asarray(input, np.float32)
    Wq = np.asarray(Wq, np.float32)
    bq = np.asarray(bq, np.float32)
    Wk = np.asarray(Wk, np.float32)
    bk = np.asarray(bk, np.float32)
    Wv = np.asarray(Wv, np.float32)
    bv = np.asarray(bv, np.float32)

    def pack_w(W):
        # [128, 2048] fp8: [c_lo, cp*1024 + s*512 + k], c = cp*256+s*128+c_lo
        Wt = (W * 32.0).T.reshape(2, 2, P, C)          # [cp, s, c_lo, k]
        return np.ascontiguousarray(
            Wt.transpose(2, 0, 1, 3).reshape(P, 2048)).astype(fp8)

    wq = pack_w(Wq)
    wk = pack_w(Wk)
    wv = pack_w(Wv)

    bqk = np.empty((P, 8), np.float32)
    for kt in range(4):
        bqk[:, kt] = 2.0 * bq[kt * P:(kt + 1) * P]
        bqk[:, 4 + kt] = 2.0 * bk[kt * P:(kt + 1) * P]

    # per-key-row power-of-2 softmax split: t_i = round(log2(2.39 n_i)/2)
    i_idx = np.arange(T)
    n_i = (T - i_idx).astype(np.float64)
    t_i = np.round(0.5 * np.log2(2.39 * n_i)).astype(np.float32)
    bexp = np.ascontiguousarray(
        (-t_i * math.log(2.0)).reshape(NT, P).T).astype(np.float32)

    pp = np.arange(P)[:, None]
    xx = np.arange(P)[None, :]
    mask = np.where(xx < pp, NEG, 0.0).astype(bf)

    bvf = np.ascontiguousarray(
        np.broadcast_to(np.tile(bv, 2).astype(bf), (P, 2 * C)))

    in_maps = []
    for b in range(B):
        in_maps.append({
            "xt": np.ascontiguousarray(input[b].T).astype(fp8),
            "wq": wq, "wk": wk, "wv": wv,
            "bqk": bqk, "bexp": bexp, "mask": mask, "bvf": bvf,
        })
    return in_maps


def kernel(input, Wq, bq, Wk, bk, Wv, bv, _trace=False):
    from concourse.bass_utils import run_bass_kernel_spmd

    nc = _get_built()
    in_maps = _make_in_maps(input, Wq, bq, Wk, bk, Wv, bv)
    res = run_bass_kernel_spmd(nc, in_maps, core_ids=list(range(NCORES)),
                               trace=_trace)
    input = np.asarray(input, np.float32)
    out = np.empty((B, T, 2 * C), np.float32)
    for b in range(B):
        out[b, :, 0:C] = input[b]
        out[b, :, C:] = res.results[b]["out"]
    if _trace:
        kernel.last_result = res
    return out


# revision 27
# speedup vs baseline: 1.2359x; 1.0524x over previous
"""Trainium2 Bass kernel for an attention block with a non-standard
(query-axis) softmax and causal mask.  fp8 DoubleRow version.

Math per batch element b (T=2048 tokens, C=K=V=512):
    q = x @ Wq.T + bq ; k = x @ Wk.T + bk ; v = x @ Wv.T + bv
    logits[j, i] = q[j] . k[i]                     (j=query, i=key)
    masked = -inf where i > j
    probs = softmax(masked / sqrt(512), axis=j)    <-- softmax over QUERY axis
    read[j] = sum_i probs[j, i] * v[i]
    out = concat(x, read)                          [T, 1024]

Distribution: pure data-parallel, batch b -> core b (8 batches, 8 cores),
weights replicated, no collectives.  The passthrough half of the output is
assembled on the host (np.concatenate); the device computes only read.

All matmuls run in fp8(e4m3) with perf_mode=DoubleRow: one instruction
contracts TWO 128-row slabs (lhsT [128,2,M], rhs [128,2,N]) at the same
rate a bf16 matmul contracts one -- 2x effective PE throughput (measured
222ns per [128out x 512free x 256contract] on HW).

Scale management (e4m3: max 240, min normal 2^-6):
  - X cast to fp8 directly (rms 1).  Weights scaled by 32 (rms 0.64); the
    1/32 is folded into the PSUM-evacuation affines.
  - Q,K stored as fp8 2q, 2k (rms 0.9); the extra 2*2 and the softmax
    1/sqrt(512) fold into the ACT exp scale.
  - The softmax normalizer 1/sum_j e spans [1/2048, 1] across key rows i.
    That 11-octave range is split evenly between the two read-matmul
    operands: E~[i,j] = e[i,j]*2^-t_i (via a static per-partition bias
    -t_i*ln2 added inside the exp) and V~[i,v] = v[i,v] / sum_j E~[i,j].
    Then E~ . V~ == probs . v exactly, and both operands sit near the
    middle of the fp8 range (t_i = round(log2(2.39*n_i)/2), n_i = 2048-i).

Engine budget (PSUM can only be read by ACT/DVE; GPSIMD is SBUF-only):
  PE ~56us (248 DR matmuls + warmups), ACT ~35us (Q/K affines via
  Identity+bias, 24 exps with accum), DVE ~39us (K affines, V bias-stt,
  triangular mask adds on the 128-wide diagonal strips, reciprocals,
  PSUM->SBUF output copies), GPSIMD ~11us (V~ scaling into fp8, part sums,
  zero strips for the even-jt diagonal pairs, nothing touching PSUM).

Phase 2 (logits+exp) and phase 3 (read) are emission-interleaved so the
in-order PE queue never waits long on the ACT exp pipeline; the last two
read rows' accumulation chains are split so only one pair of matmuls
remains after the final exp.
"""

import math

import numpy as np
import ml_dtypes

P = 128
B, T, C = 8, 2048, 512
NT = T // P     # 16 row tiles
NCORES = 8
NEG = -1e30
SEXP = 1.0 / (4.0 * math.sqrt(512.0))

_BUILT = None


def _build_nc():
    import concourse.mybir as mybir
    import concourse.tile as tile
    from concourse import bacc

    f32 = mybir.dt.float32
    bf16 = mybir.dt.bfloat16
    fp8 = mybir.dt.float8e4
    AF = mybir.ActivationFunctionType
    ALU = mybir.AluOpType
    DR = mybir.MatmulPerfMode.DoubleRow

    nc = bacc.Bacc("TRN2", target_bir_lowering=False, debug=False,
                   num_devices=NCORES)

    xt_d = nc.dram_tensor("xt", [C, T], fp8, kind="ExternalInput")
    wq_d = nc.dram_tensor("wq", [P, 2048], fp8, kind="ExternalInput")
    wk_d = nc.dram_tensor("wk", [P, 2048], fp8, kind="ExternalInput")
    wv_d = nc.dram_tensor("wv", [P, 2048], fp8, kind="ExternalInput")
    bqk_d = nc.dram_tensor("bqk", [P, 8], f32, kind="ExternalInput")
    bexp_d = nc.dram_tensor("bexp", [P, NT], f32, kind="ExternalInput")
    mask_d = nc.dram_tensor("mask", [P, P], bf16, kind="ExternalInput")
    bvf_d = nc.dram_tensor("bvf", [P, 2 * C], bf16, kind="ExternalInput")
    out_d = nc.dram_tensor("out", [T, C], f32, kind="ExternalOutput")

    with tile.TileContext(nc) as tc:
        with (
            tc.tile_pool(name="const", bufs=1) as cpool,
            tc.tile_pool(name="w", bufs=1) as wpool,
            tc.tile_pool(name="xt", bufs=1) as xtpool,
            tc.tile_pool(name="qt", bufs=1) as qtpool,
            tc.tile_pool(name="kt", bufs=1) as ktpool,
            tc.tile_pool(name="v", bufs=1) as vpool,
            tc.tile_pool(name="vp", bufs=1) as vppool,
            tc.tile_pool(name="et", bufs=1) as etpool,
            tc.tile_pool(name="small", bufs=16) as spool,
            tc.tile_pool(name="ostage", bufs=3) as ospool,
            tc.tile_pool(name="psq", bufs=2, space="PSUM") as psq,   # 2x1024
            tc.tile_pool(name="psl", bufs=2, space="PSUM") as psl,   # 2x1024
        ):
            # --- loads ---
            # dma_start issue costs ~0.6us of engine time each, so the
            # issues are split across both HWDGE engines (sync + scalar)
            # and ordered most-urgent-first on each.
            xt2 = [xtpool.tile([P, 2, T], fp8, name=f"xt{cp}", tag=f"xt{cp}")
                   for cp in range(2)]
            for cp in range(2):
                for s in range(2):
                    r0 = (2 * cp + s) * P
                    nc.sync.dma_start(xt2[cp][:, s, :], xt_d[r0:r0 + P, :])
            wq2 = []
            for cp in range(2):
                t_ = wpool.tile([P, 2, C], fp8, name=f"wq{cp}", tag=f"wq{cp}")
                nc.scalar.dma_start(t_[:, :, :],
                                    wq_d[:, cp * 1024:(cp + 1) * 1024])
                wq2.append(t_)
            wk2 = []
            for cp in range(2):
                t_ = wpool.tile([P, 2, C], fp8, name=f"wk{cp}", tag=f"wk{cp}")
                nc.scalar.dma_start(t_[:, :, :],
                                    wk_d[:, cp * 1024:(cp + 1) * 1024])
                wk2.append(t_)
            bqk_t = cpool.tile([P, 8], f32, name="bqk_t")
            nc.scalar.dma_start(bqk_t[:], bqk_d[:])
            wv2 = []
            for cp in range(2):
                t_ = wpool.tile([P, 2, C], fp8, name=f"wv{cp}", tag=f"wv{cp}")
                nc.sync.dma_start(t_[:, :, :],
                                  wv_d[:, cp * 1024:(cp + 1) * 1024])
                wv2.append(t_)
            bvf_t = cpool.tile([P, 2 * C], bf16, name="bvf_t")
            nc.sync.dma_start(bvf_t[:], bvf_d[:])
            bexp_t = cpool.tile([P, NT], f32, name="bexp_t")
            nc.sync.dma_start(bexp_t[:], bexp_d[:])
            mask_t = cpool.tile([P, P], bf16, name="mask_t")
            nc.sync.dma_start(mask_t[:], mask_d[:])

            # PE warm-up: junk bf16 matmuls with no DMA dependency to start
            # the HAM clock-gate ramp during the load window.
            warm_src = cpool.tile([P, C + P], bf16, name="warm_src")
            nc.vector.memset(warm_src[:], 0.0)
            ps_warm = psq.tile([P, 1024], f32, name="ps_warm", tag="psq")
            for _ in range(7):
                nc.tensor.matmul(ps_warm[:, 0:512], warm_src[:, C:C + P],
                                 warm_src[:, 0:C], start=True, stop=True)

            # --- Phase 1a: Q^T, K^T in [k, 2, t] fp8 pair layout ---
            # Q^T psum = 32*q_raw; evac: *1/16 + 2*b -> fp8 2q (rms ~0.9).
            # Two 512-col chunks share one [128,1024] psum tile so a single
            # 1024-wide affine evacuates both (half the evac ops, and the
            # PE stays dense enough to hold the HAM clock at full rate).
            # jcp-outer order: the first half of phase 1a touches only
            # X^T[:, 0:1024], hiding the rest of the X^T load.
            qt2 = [qtpool.tile([P, 2, T], fp8, name=f"qt{kp}", tag=f"qt{kp}")
                   for kp in range(2)]
            kt2 = [ktpool.tile([P, 2, T], fp8, name=f"kt{kp}", tag=f"kt{kp}")
                   for kp in range(2)]
            for jcp in range(2):
                jp = slice(jcp * 1024, (jcp + 1) * 1024)
                for kt in range(4):
                    ksl = slice(kt * P, (kt + 1) * P)
                    ps = psq.tile([P, 1024], f32, name="psq1", tag="psq")
                    for h in range(2):
                        js = slice((2 * jcp + h) * 512,
                                   (2 * jcp + h + 1) * 512)
                        for cp in range(2):
                            nc.tensor.matmul(ps[:, h * 512:(h + 1) * 512],
                                             wq2[cp][:, :, ksl],
                                             xt2[cp][:, :, js],
                                             start=(cp == 0), stop=(cp == 1),
                                             perf_mode=DR)
                    nc.scalar.activation(qt2[kt // 2][:, kt % 2, jp], ps[:],
                                         AF.Identity,
                                         bias=bqk_t[:, kt:kt + 1],
                                         scale=1.0 / 16.0)
                    ps2 = psq.tile([P, 1024], f32, name="psq2", tag="psq")
                    for h in range(2):
                        js = slice((2 * jcp + h) * 512,
                                   (2 * jcp + h + 1) * 512)
                        for cp in range(2):
                            nc.tensor.matmul(ps2[:, h * 512:(h + 1) * 512],
                                             wk2[cp][:, :, ksl],
                                             xt2[cp][:, :, js],
                                             start=(cp == 0), stop=(cp == 1),
                                             perf_mode=DR)
                    if kt % 2 == 0:
                        nc.vector.tensor_scalar(kt2[kt // 2][:, kt % 2, jp],
                                                ps2[:], 1.0 / 16.0,
                                                bqk_t[:, 4 + kt:5 + kt],
                                                ALU.mult, ALU.add)
                    else:
                        nc.scalar.activation(kt2[kt // 2][:, kt % 2, jp],
                                             ps2[:], AF.Identity,
                                             bias=bqk_t[:, 4 + kt:5 + kt],
                                             scale=1.0 / 16.0)

            # --- Phase 1b: V in natural [t, v] layout, bf16 (true scale) ---
            # Paired (two token tiles per psum/evac) and emitted interleaved
            # into the phase-2 loop so the DVE evacuations don't bunch up in
            # front of phase 2's mask adds.
            v2_sb = [vpool.tile([P, 2 * C], bf16, name=f"v{tp}", tag=f"v{tp}")
                     for tp in range(8)]

            def p1b_pair(tp):
                psV = psq.tile([P, 1024], f32, name="psv", tag="psq")
                for h in range(2):
                    tsl = slice((2 * tp + h) * P, (2 * tp + h + 1) * P)
                    for cp in range(2):
                        nc.tensor.matmul(psV[:, h * 512:(h + 1) * 512],
                                         xt2[cp][:, :, tsl],
                                         wv2[cp][:, :, :],
                                         start=(cp == 0), stop=(cp == 1),
                                         perf_mode=DR)
                nc.vector.scalar_tensor_tensor(v2_sb[tp][:], psV[:],
                                               1.0 / 32.0, bvf_t[:],
                                               ALU.mult, ALU.add)

            # E~ pair tiles [i_lo, s, j]; key row i = 256r + 128s + i_lo.
            et2 = [etpool.tile([P, 2, T], fp8, name=f"et{r}", tag=f"et{r}")
                   for r in range(8)]
            # Zero the garbage diagonal strips read by even-jt read chains:
            # pair r = jt/2 slab 1 is key tile it = jt+1, whose columns
            # [128jt, 128(jt+1)) are fully masked and never written by exp.
            for jt in range(0, NT, 2):
                nc.vector.memset(et2[jt // 2][:, 1, jt * P:(jt + 1) * P], 0.0)
            vp2 = [vppool.tile([P, 2, C], fp8, name=f"vp{r}", tag=f"vp{r}")
                   for r in range(8)]

            # --- Phase 2 per key tile: masked logits + exp + row sums ---
            def p2_it(it):
                isl = slice(it * P, (it + 1) * P)
                parts = []
                w0 = it * P
                # 1024-wide psum chunks, 1024-aligned in absolute column
                # space; each 512-aligned matmul subchunk stays in one bank.
                for base in range(w0 // 1024 * 1024, T, 1024):
                    lo = max(w0, base)
                    hi = base + 1024
                    pslt = psl.tile([P, 1024], f32, name="psl", tag="psl")
                    s0 = lo
                    while s0 < hi:
                        s1 = min((s0 // 512 + 1) * 512, hi)
                        for cp in range(2):
                            nc.tensor.matmul(pslt[:, s0 - base:s1 - base],
                                             kt2[cp][:, :, isl],
                                             qt2[cp][:, :, s0:s1],
                                             start=(cp == 0), stop=(cp == 1),
                                             perf_mode=DR)
                        s0 = s1
                    if lo == w0:  # triangular mask on the diagonal strip
                        nc.vector.tensor_add(pslt[:, lo - base:lo - base + P],
                                             pslt[:, lo - base:lo - base + P],
                                             mask_t[:])
                    part = spool.tile([P, 1], f32, name="part", tag="part")
                    nc.scalar.activation(et2[it // 2][:, it % 2, lo:hi],
                                         pslt[:, lo - base:1024], AF.Exp,
                                         bias=bexp_t[:, it:it + 1],
                                         scale=SEXP, accum_out=part[:])
                    parts.append(part)
                s_t = parts[0]
                for p_ in parts[1:]:
                    ns = spool.tile([P, 1], f32, name="s_t", tag="s_t")
                    nc.vector.tensor_add(ns[:], s_t[:], p_[:])
                    s_t = ns
                r_t = spool.tile([P, 1], f32, name="r_t", tag="r_t")
                nc.vector.reciprocal(r_t[:], s_t[:])
                nc.vector.tensor_scalar_mul(
                    vp2[it // 2][:, it % 2, :],
                    v2_sb[it // 2][:, (it % 2) * 512:(it % 2 + 1) * 512],
                    r_t[:, 0:1])

            # --- Phase 3: read rows, two query tiles (jt=2m, 2m+1) share
            # one [128,1024] psum tile and one paired evacuation.  Both
            # column chains accumulate over the same r = 0..m pair range.
            ps_g = {}

            def p3_group(m, rlo, rhi):
                if m not in ps_g:
                    ps_g[m] = psq.tile([P, 1024], f32, name="psg", tag="psq")
                for r in range(rlo, rhi):
                    for h in range(2):
                        jsl = slice((2 * m + h) * P, (2 * m + h + 1) * P)
                        nc.tensor.matmul(ps_g[m][:, h * 512:(h + 1) * 512],
                                         et2[r][:, :, jsl], vp2[r][:, :, :],
                                         start=(r == 0), stop=(r == m),
                                         perf_mode=DR)

            def p3_out(m, half=None):
                # all loads are done by now, so the idle sync HWDGE path
                # carries the output (gpsimd's DGE drain is expensive)
                ost = ospool.tile([P, 1024], f32, name="ost", tag="ost")
                r0 = 2 * m * P
                if half is None:
                    if m % 2 == 0:
                        nc.scalar.activation(ost[:], ps_g[m][:], AF.Copy)
                    else:
                        nc.vector.tensor_copy(ost[:], ps_g[m][:])
                    nc.sync.dma_start(out_d[r0:r0 + P, :], ost[:, 0:512])
                    nc.sync.dma_start(out_d[r0 + P:r0 + 2 * P, :],
                                      ost[:, 512:1024])
                else:
                    h = half
                    nc.vector.tensor_copy(ost[:, h * 512:(h + 1) * 512],
                                          ps_g[m][:, h * 512:(h + 1) * 512])
                    nc.sync.dma_start(out_d[r0 + h * P:r0 + (h + 1) * P, :],
                                      ost[:, h * 512:(h + 1) * 512])

            p1b_pair(0)
            p1b_pair(1)
            for k in range(14):
                if k % 2 == 0 and k // 2 + 2 < 8:
                    p1b_pair(k // 2 + 2)
                p2_it(k)
                if k >= 5 and k % 2 == 1:
                    m = (k - 5) // 2
                    p3_group(m, 0, m + 1)
                    p3_out(m)
            # k=13 done (groups 0..4 emitted): 5,6 fully ready; 7 all but r=7
            p3_group(5, 0, 6)
            p3_out(5)
            p3_group(6, 0, 7)
            p3_out(6)
            p3_group(7, 0, 7)
            p2_it(14)
            p2_it(15)
            # final pair: evacuate each half right behind its last matmul
            jsl = slice(14 * P, 15 * P)
            nc.tensor.matmul(ps_g[7][:, 0:512], et2[7][:, :, jsl],
                             vp2[7][:, :, :], start=False, stop=True,
                             perf_mode=DR)
            p3_out(7, half=0)
            jsl = slice(15 * P, 16 * P)
            nc.tensor.matmul(ps_g[7][:, 512:1024], et2[7][:, :, jsl],
                             vp2[7][:, :, :], start=False, stop=True,
                             perf_mode=DR)
            p3_out(7, half=1)

    nc.compile()
    return nc


def _get_built():
    global _BUILT
    if _BUILT is None:
        _BUILT = _build_nc()
    return _BUILT


def _make_in_maps(input, Wq, bq, Wk, bk, Wv, bv):
    fp8 = ml_dtypes.float8_e4m3
    bf = ml_dtypes.bfloat16

    input = np.asarray(input, np.float32)
    Wq = np.asarray(Wq, np.float32)
    bq = np.asarray(bq, np.float32)
    Wk = np.asarray(Wk, np.float32)
    bk = np.asarray(bk, np.float32)
    Wv = np.asarray(Wv, np.float32)
    bv = np.asarray(bv, np.float32)

    def pack_w(W):
        # [128, 2048] fp8: [c_lo, cp*1024 + s*512 + k], c = cp*256+s*128+c_lo
        Wt = (W * 32.0).T.reshape(2, 2, P, C)          # [cp, s, c_lo, k]
        return np.ascontiguousarray(
            Wt.transpose(2, 0, 1, 3).reshape(P, 2048)).astype(fp8)

    wq = pack_w(Wq)
    wk = pack_w(Wk)
    wv = pack_w(Wv)

    bqk = np.empty((P, 8), np.float32)
    for kt in range(4):
        bqk[:, kt] = 2.0 * bq[kt * P:(kt + 1) * P]
        bqk[:, 4 + kt] = 2.0 * bk[kt * P:(kt + 1) * P]

    # per-key-row power-of-2 softmax split: t_i = round(log2(2.39 n_i)/2)
    i_idx = np.arange(T)
    n_i = (T - i_idx).astype(np.float64)
    t_i = np.round(0.5 * np.log2(2.39 * n_i)).astype(np.float32)
    bexp = np.ascontiguousarray(
        (-t_i * math.log(2.0)).reshape(NT, P).T).astype(np.float32)

    pp = np.arange(P)[:, None]
    xx = np.arange(P)[None, :]
    mask = np.where(xx < pp, NEG, 0.0).astype(bf)

    bvf = np.ascontiguousarray(
        np.broadcast_to(np.tile(bv, 2).astype(bf), (P, 2 * C)))

    in_maps = []
    for b in range(B):
        in_maps.append({
            "xt": np.ascontiguousarray(input[b].T).astype(fp8),
            "wq": wq, "wk": wk, "wv": wv,
            "bqk": bqk, "bexp": bexp, "mask": mask, "bvf": bvf,
        })
    return in_maps


def kernel(input, Wq, bq, Wk, bk, Wv, bv, _trace=False):
    from concourse.bass_utils import run_bass_kernel_spmd

    nc = _get_built()
    in_maps = _make_in_maps(input, Wq, bq, Wk, bk, Wv, bv)
    res = run_bass_kernel_spmd(nc, in_maps, core_ids=list(range(NCORES)),
                               trace=_trace)
    input = np.asarray(input, np.float32)
    out = np.empty((B, T, 2 * C), np.float32)
    for b in range(B):
        out[b, :, 0:C] = input[b]
        out[b, :, C:] = res.results[b]["out"]
    if _trace:
        kernel.last_result = res
    return out
